# revision 4
# baseline (speedup 1.0000x reference)
"""Trainium2 Bass kernel for nn_LogisticModel.

logp[b,t] = -0.5 * z^2 - (log(NOISE) + 0.5*log(2*pi))
  where z = (x[b,t] - DECAY*x[b,t-1] - sigmoid(GAIN*s[b,t])) / NOISE, x[b,-1] = 0.

Pure data parallel: batch 4096 rows split 8 ways (512 rows/core).
Per core: 4 row-blocks x 2 col-blocks of [128, 4096] tiles; x is loaded
with a one-column halo so the time shift is a free SBUF offset.

The kernel is HBM-bound (reads 32 MiB f32 inputs + writes the output per
core), so the optimizations are pure traffic/efficiency ones:
  - output is written as bf16 (host upcasts): halves write traffic.
  - intermediates after the first DVE op are bf16: 2x DVE/ACT rate.
  - both input streams ride the two HWDGE rings (s on SP, x on ACT);
    SWDGE/gpsimd is avoided entirely (Q7 descriptor emission + ring
    drain cost ~12us on HW and its per-dma setup is ~2x HWDGE's).
"""

import math

import numpy as np

import concourse.bass as bass
import concourse.bacc as bacc
import concourse.tile as tile
from concourse import mybir
from concourse import bass_utils

GAIN = 2.0
DECAY = 0.9
NOISE = 0.1
BATCH, T = 4096, 8192
N_CORES = 8
ROWS_PER_CORE = BATCH // N_CORES  # 512
P = 128                           # SBUF partitions
W = 4096                          # free-dim tile width
NEG_C = -(math.log(NOISE) + 0.5 * math.log(2.0 * math.pi))  # +1.3836466...

_nc_cache = None


def _build_nc():
    # Bacc (not raw Bass): its finalize() runs generate_event_semaphores,
    # which splits multi-wait sync into the <=1-wait-per-instruction form
    # walrus requires ("Too many sync wait commands" otherwise).
    nc = bacc.Bacc("TRN2", target_bir_lowering=False, detect_race_conditions=False)
    f32 = mybir.dt.float32
    bf16 = mybir.dt.bfloat16
    s = nc.dram_tensor("s", [ROWS_PER_CORE, T], f32, kind="ExternalInput")
    x = nc.dram_tensor("x", [ROWS_PER_CORE, T], f32, kind="ExternalInput")
    out = nc.dram_tensor("out", [ROWS_PER_CORE, T], bf16, kind="ExternalOutput")

    n_rblk = ROWS_PER_CORE // P  # 4
    n_cblk = T // W              # 2

    with tile.TileContext(nc) as tc:
        with (
            tc.tile_pool(name="in", bufs=4) as in_pool,
            tc.tile_pool(name="out", bufs=3) as out_pool,
            tc.tile_pool(name="tmp", bufs=2) as tmp_pool,
        ):
            for idx in range(n_rblk * n_cblk):
                    r, j = divmod(idx, n_cblk)
                    rs = bass.ts(r, P)
                    cs = bass.ts(j, W)

                    s_t = in_pool.tile([P, W], f32, tag="s_t")
                    nc.sync.dma_start(s_t[:], s[rs, cs])

                    # x tile with 1-col halo: col 0 = x[t-1] of first element
                    x_t = in_pool.tile([P, W + 1], f32, tag="x_t")
                    if j == 0:
                        nc.vector.memset(x_t[:, 0:1], 0.0)
                        nc.scalar.dma_start(x_t[:, 1 : W + 1], x[rs, 0:W])
                    else:
                        nc.scalar.dma_start(x_t[:], x[rs, j * W - 1 : (j + 1) * W])

                    # b = sigmoid(GAIN * s)           [ACT, f32 -> bf16]
                    b_t = tmp_pool.tile([P, W], bf16, tag="b_t")
                    nc.scalar.activation(
                        b_t[:], s_t[:], mybir.ActivationFunctionType.Sigmoid,
                        scale=GAIN,
                    )
                    # v = (x_prev * -DECAY) + x_cur   [DVE, f32 -> bf16]
                    v_t = tmp_pool.tile([P, W], bf16, tag="v_t")
                    nc.vector.scalar_tensor_tensor(
                        v_t[:], x_t[:, 0:W], -DECAY, x_t[:, 1 : W + 1],
                        mybir.AluOpType.mult, mybir.AluOpType.add,
                    )
                    # f = v - b, in place into v      [DVE, all-bf16 2x mode]
                    nc.vector.tensor_sub(v_t[:], v_t[:], b_t[:])
                    # g = (f / NOISE)^2 = z^2, into b_t [ACT, bf16]
                    nc.scalar.activation(
                        b_t[:], v_t[:], mybir.ActivationFunctionType.Square,
                        scale=1.0 / NOISE,
                    )
                    # out = -0.5*g + NEG_C            [DVE, all-bf16 2x mode]
                    o_t = out_pool.tile([P, W], bf16, tag="o_t")
                    nc.vector.tensor_scalar(
                        o_t[:], b_t[:], -0.5, NEG_C,
                        mybir.AluOpType.mult, mybir.AluOpType.add,
                    )
                    # alternate stores across the two HWDGE rings so the
                    # final drain overlaps both queues
                    (nc.sync if idx % 2 == 0 else nc.scalar).dma_start(
                        out[rs, cs], o_t[:]
                    )
    # Bacc defers register assignment to alloc_regs() inside finalize();
    # run_bass_kernel_spmd doesn't call it for prebuilt modules.
    nc.finalize()
    return nc


def _get_nc():
    global _nc_cache
    if _nc_cache is None:
        _nc_cache = _build_nc()
    return _nc_cache


def run_spmd(s, x, **kw):
    """Shard rows across 8 cores, run, gather. Returns (out, BassKernelResults)."""
    s = np.ascontiguousarray(np.asarray(s, dtype=np.float32))
    x = np.ascontiguousarray(np.asarray(x, dtype=np.float32))
    assert s.shape == (BATCH, T) and x.shape == (BATCH, T)
    in_maps = [
        {
            "s": s[i * ROWS_PER_CORE : (i + 1) * ROWS_PER_CORE],
            "x": x[i * ROWS_PER_CORE : (i + 1) * ROWS_PER_CORE],
        }
        for i in range(N_CORES)
    ]
    res = bass_utils.run_bass_kernel_spmd(
        _get_nc(), in_maps, core_ids=list(range(N_CORES)), **kw
    )
    out = np.concatenate(
        [np.asarray(m["out"]).astype(np.float32) for m in res.results], axis=0
    )
    return out, res


def kernel(s, x):
    out, _ = run_spmd(s, x)
    return out
